# revision 23
# baseline (speedup 1.0000x reference)
"""Multi-head latent attention (MLA) Trainium2 kernel, 8-core SPMD.

Sharding: cores split into 2 batch-groups of 4 (cores 0-3 = batch 0,
4-7 = batch 1). Within a group, core w owns token shard [512w, 512w+512)
of its batch and heads {4w..4w+3}.

  - phase A1 (token-parallel): k/v latents + RoPE'd pos_k for the OWN
    token shard; group AllGather (partition-major layout).
  - phase A2 (replicated, overlaps the AllGather): q-latents (lq) for ALL
    batch tokens computed locally - removing lq from the AllGather shrinks
    it by a third, and the redundant compute hides inside the gather.
  - phase B: up-projections. The q-side (qT, RoPE'd positional queries)
    depends only on local lq, so it also overlaps the AllGather; the
    k/v side consumes gathered latents.
    RoPE rotation via pre-permuted weight copies:
    rope(u) = u*cos + perm(u)*sin_signed.
  - phase C: attention in transposed orientation scoresT[k, q]:
    pT = exp(scoresT*scale) feeds attnT = v^T @ pT directly; denominators
    via ones-column matmul; max-free softmax (scores bounded, fp32 exp).
  - phase D (interleaved into C, span-outer): after C finishes q-span u,
    the partial o_proj for span u's tokens over local heads runs
    immediately (+ b_o/4 so the group sum restores the bias once) and a
    per-span ReduceScatter hands each core its 128-token slice of the
    summed span. Only the last span's RS is exposed at the end.
All matmul operands bf16, fp32 PSUM accumulation. Host assembles shards.
"""
import numpy as np
import ml_dtypes

import concourse.bacc as bacc
import concourse.mybir as mybir
import concourse.tile as tile
from concourse.bass_utils import run_bass_kernel_spmd
from concourse.tile import add_dep_helper


def _dep(a, b, reason):
    add_dep_helper(getattr(a, "ins", a), getattr(b, "ins", b), sync=False,
                   reason=reason)

F32 = mybir.dt.float32
BF16 = mybir.dt.bfloat16
AF = mybir.ActivationFunctionType
OP = mybir.AluOpType
BF = ml_dtypes.bfloat16

MODEL = 2048
LATENT = 512
NH = 16
HD = 128          # head dim (main)
PHD = 64          # positional head dim
THETA = 50000.0
B = 2
S = 2048
T = B * S
NC = 8
G = 4             # cores per batch-group
TS = T // NC      # 512 tokens per core shard
HC = NH // G      # 4 heads per core
SCALE = 1.0 / float(np.sqrt(HD + PHD))

LJ = LATENT // 128                # 4 l-chunks per latent
NLT = 3 * LJ + 1                  # 13 w_cat column tiles
AGW = 4 * TS + TS // 2            # 2304: lv(4) + packed posk
NU = S // TS                      # 4 q spans per batch

# bias views into bcon: cols [0:13] b_cat, then q heads, k heads, qpos packs
BQ0, BK0, BP0 = NLT, NLT + HC, NLT + 2 * HC
# wup col layout per j-chunk (stride JW)
JW = 1792
WQ, WK, WV, WP = 0, 512, 1024, 1536

_ROT = np.r_[32:64, 0:32]

# C/D processing chunks (q-start, width). The tail is two 256-token chunks
# so the final ReduceScatter is half-size and the penultimate one hides
# under the last chunk's compute.
CHUNKS = [(0, 512), (512, 512), (1024, 512), (1536, 256), (1792, 256)]

_CACHE = {}


def _build():
    nc = bacc.Bacc("TRN2", target_bir_lowering=False, debug=False,
                   num_devices=NC)

    xT = nc.dram_tensor("xT", [128, 16 * TS], BF16, kind="ExternalInput")
    xTb = nc.dram_tensor("xTb", [128, 4 * 16 * TS], BF16,
                         kind="ExternalInput")
    w_catp = nc.dram_tensor("w_catp", [128, NLT * 2048], BF16,
                            kind="ExternalInput")
    wup = nc.dram_tensor("wup", [128, LJ * JW], BF16, kind="ExternalInput")
    wolp = nc.dram_tensor("wolp", [128, HC * MODEL], BF16,
                          kind="ExternalInput")
    bcon = nc.dram_tensor("bcon", [128, BP0 + 4], F32, kind="ExternalInput")
    bvb = nc.dram_tensor("bvb", [128, HC * HD], BF16, kind="ExternalInput")
    bob = nc.dram_tensor("bob", [128, MODEL], BF16, kind="ExternalInput")
    sc2 = nc.dram_tensor("sc2", [128, 2 * S], BF16, kind="ExternalInput")
    sc_sh = nc.dram_tensor("sc_sh", [128, TS], F32, kind="ExternalInput")
    tri = nc.dram_tensor("tri", [128, 128], BF16, kind="ExternalInput")
    out_sh = nc.dram_tensor("out_sh", [TS, MODEL], BF16,
                            kind="ExternalOutput")

    groups = [[0, 1, 2, 3], [4, 5, 6, 7]]

    with tile.TileContext(nc) as tc:
        with (
            tc.tile_pool(name="const", bufs=1) as cpool,
            tc.tile_pool(name="psum", bufs=1, space="PSUM") as pspool,
            tc.tile_pool(name="dram", bufs=1, space="DRAM") as dram,
        ):
            # ---------- constants (phase-A-critical first) ----------
            bcon_sb = cpool.tile([128, BP0 + 4], F32, tag="bcon")
            sc_sh_sb = cpool.tile([128, TS], F32, tag="scsh")
            bvb_sb = cpool.tile([128, HC * HD], BF16, tag="bvb")
            bob_sb = cpool.tile([128, MODEL], BF16, tag="bob")
            sc2_sb = cpool.tile([128, 2 * S], BF16, tag="sc2")
            tri_sb = cpool.tile([128, 128], BF16, tag="tri")
            wup_sb = cpool.tile([128, LJ * JW], BF16, tag="wup")
            ones_col = cpool.tile([128, 1], BF16, tag="onesc")
            ones_row = cpool.tile([1, 128], BF16, tag="onesr")

            ag_in = dram.tile([128, AGW], BF16)
            ag_out = dram.tile([G * 128, AGW], BF16)
            rs_in = [dram.tile([W, MODEL], BF16, name=f"rsin{ci}")
                     for ci, (q0, W) in enumerate(CHUNKS)]
            rs_out = [dram.tile([W // G, MODEL], BF16, name=f"rsout{ci}")
                      for ci, (q0, W) in enumerate(CHUNKS)]

            with (
                tc.tile_pool(name="phA", bufs=1) as apool,
                tc.tile_pool(name="phAw", bufs=1) as awork,
            ):
                _sid = nc.enter_named_scope("A1", False)[0]
                # ------- phase A1: v/posk latents on own token shard -------
                # lat_sb shares the CD-phase st ring (disjoint lifetimes)
                lat_sb = awork.tile([128, AGW], BF16, tag="st", bufs=2,
                                    name="latA")
                xs = awork.tile([128, 16 * TS], BF16, tag="xs", bufs=1,
                                name="xself")
                # interleave weight/x loads so the first matmul can issue
                # after ~2 transfers (the DMA stream is serial); bcon/scsh
                # slot in before the ACT/DVE consumers need them
                wA1 = []
                for jj in range(5):
                    j = 8 + jj          # w_cat tiles 8..12 (lv, posk)
                    wj = awork.tile([128, 2048], BF16, tag="wA", bufs=4,
                                    name=f"wA{j}")
                    wA1.append(wj)
                order = [("w", 0), ("x", 0), ("b", 0), ("x", 1), ("w", 1),
                         ("b", 1), ("x", 2), ("x", 3), ("w", 2), ("w", 3),
                         ("w", 4)]
                for kind, i in order:
                    if kind == "w":
                        nc.sync.dma_start(
                            out=wA1[i][:],
                            in_=w_catp.ap()[:, 2048 * (8 + i):2048 * (9 + i)])
                    elif kind == "b":
                        if i == 0:
                            nc.sync.dma_start(out=bcon_sb[:], in_=bcon.ap())
                        else:
                            nc.sync.dma_start(out=sc_sh_sb[:],
                                              in_=sc_sh.ap())
                    else:
                        nc.sync.dma_start(
                            out=xs[:, 4 * TS * i:4 * TS * (i + 1)],
                            in_=xT.ap()[:, 4 * TS * i:4 * TS * (i + 1)])
                nc.vector.memset(ones_col[:], 1.0)
                nc.vector.memset(ones_row[:], 1.0)
                for jj in range(5):
                    j = 8 + jj
                    wj = wA1[jj]
                    ps = pspool.tile([128, TS], F32, tag="psA", bufs=3,
                                     name=f"psA{j}")
                    for m in range(16):
                        nc.tensor.matmul(
                            ps[:], wj[:, 128 * m:128 * (m + 1)],
                            xs[:, TS * m:TS * (m + 1)],
                            start=(m == 0), stop=(m == 15))
                    if j < 12:
                        nc.scalar.activation(
                            lat_sb[:, TS * jj:TS * (jj + 1)], ps[:],
                            AF.Identity, bias=bcon_sb[:, j:j + 1])
                    else:
                        # posk pack: rows 0:64 raw, 64:128 pre-rotated; RoPE.
                        # t3=(raw+b)*cos, t4=(rot+b_rot)*sin_signed (PSUM in0
                        # exempts the equal-base SBUF rule)
                        t3 = awork.tile([PHD, TS], F32, tag="qpt", bufs=2,
                                        name="pk3")
                        t4 = awork.tile([PHD, TS], F32, tag="qpt", bufs=2,
                                        name="pk4")
                        nc.vector.scalar_tensor_tensor(
                            t3[:], ps[0:PHD, :], bcon_sb[0:PHD, j:j + 1],
                            sc_sh_sb[0:PHD, :], OP.add, OP.mult)
                        nc.vector.scalar_tensor_tensor(
                            t4[:], ps[PHD:128, :], bcon_sb[PHD:128, j:j + 1],
                            sc_sh_sb[PHD:128, :], OP.add, OP.mult)
                        H = TS // 2
                        nc.vector.tensor_tensor(
                            lat_sb[0:PHD, 4 * TS:4 * TS + H],
                            t3[:, 0:H], t4[:, 0:H], OP.add)
                        nc.vector.tensor_tensor(
                            lat_sb[PHD:128, 4 * TS:4 * TS + H],
                            t3[:, H:TS], t4[:, H:TS], OP.add)
                # interleave resident lq weight loads with span-0 x so
                # AB's first matmuls have operands the moment A1 drains
                wAq_r = []
                for j in range(LJ):      # resident lq blocks 0..3
                    t_ = apool.tile([128, 2048], BF16, tag=f"wAr{j}",
                                    name=f"wAr{j}")
                    wAq_r.append(t_)
                xb0 = awork.tile([128, 16 * TS], BF16, tag="xb", bufs=1,
                                 name="xb0")
                order = [("w", 0), ("x", 0), ("x", 1), ("w", 1), ("x", 2),
                         ("x", 3), ("w", 2), ("w", 3)]
                for kind, i in order:
                    if kind == "w":
                        nc.sync.dma_start(
                            out=wAq_r[i][:],
                            in_=w_catp.ap()[:, 2048 * i:2048 * (i + 1)])
                    else:
                        nc.sync.dma_start(
                            out=xb0[:, 4 * TS * i:4 * TS * (i + 1)],
                            in_=xTb.ap()[:, 4 * TS * i:4 * TS * (i + 1)])
                nc.leave_named_scope("A1", _sid, False)

                # ag_in's SEQ wait (on lat_sb) intentionally sits after the
                # span-0 loads; remaining loads overlap the AllGather
                nc.sync.dma_start(out=ag_in[:], in_=lat_sb[:])
                nc.sync.dma_start(out=wup_sb[:], in_=wup.ap())
                nc.sync.dma_start(out=sc2_sb[:], in_=sc2.ap())
                nc.sync.dma_start(out=bvb_sb[:], in_=bvb.ap())
                nc.sync.dma_start(out=tri_sb[:], in_=tri.ap())
                nc.sync.dma_start(out=bob_sb[:], in_=bob.ap())

                nc.gpsimd.collective_compute(
                    "AllGather", OP.bypass,
                    ins=[ag_in.opt()], outs=[ag_out.opt()],
                    replica_groups=groups)

                # ---------- phases B+C+D (same pools; no boundary) ----
                bpool, bwork = apool, awork
                qT = [bpool.tile([128, S], BF16, tag=f"qT{h}", name=f"qT{h}")
                      for h in range(HC)]
                kT = [bpool.tile([128, S], BF16, tag=f"kT{h}", name=f"kT{h}")
                      for h in range(HC)]
                qpp = [bpool.tile([128, S], BF16, tag=f"qpp{p}",
                                  name=f"qpp{p}") for p in range(2)]
                posk2 = bpool.tile([128, S], BF16, tag="posk2", name="posk2")
                v_sb = [[bpool.tile([128, HD], BF16, tag=f"v{h}_{tt}",
                                    name=f"v{h}_{tt}")
                         for tt in range(S // 128)] for h in range(HC)]

                _sid = nc.enter_named_scope("AB", False)[0]
                # --- replicated q/k latents + up-projections, per span; all
                # of this is AG-independent and fills the gather window ---
                last_ab_dma = None
                for s in range(4):
                    cols = slice(TS * s, TS * (s + 1))
                    if s == 0:
                        xb = xb0
                    else:
                        xb = awork.tile([128, 16 * TS], BF16, tag="xb",
                                        bufs=1, name=f"xb{s}")
                        for ch in range(4):
                            last_ab_dma = nc.sync.dma_start(
                                out=xb[:, 4 * TS * ch:4 * TS * (ch + 1)],
                                in_=xTb.ap()[:, 8192 * s + 4 * TS * ch:
                                             8192 * s + 4 * TS * (ch + 1)])
                    l2 = []
                    for j in range(8):          # lq blocks 0..3, lk 4..7
                        if j < LJ:
                            wj = wAq_r[j]
                        else:
                            wj = awork.tile([128, 2048], BF16, tag="wA",
                                            bufs=4, name=f"wAq{s}{j}")
                            nc.sync.dma_start(
                                out=wj[:],
                                in_=w_catp.ap()[:, 2048 * j:2048 * (j + 1)])
                        ps = pspool.tile([128, TS], F32, tag="psA", bufs=3,
                                         name=f"psq{s}{j}")
                        for m in range(16):
                            nc.tensor.matmul(
                                ps[:], wj[:, 128 * m:128 * (m + 1)],
                                xb[:, TS * m:TS * (m + 1)],
                                start=(m == 0), stop=(m == 15))
                        lt = bwork.tile([128, TS], BF16, tag=f"l2_{j}",
                                        bufs=1, name=f"l2_{s}{j}")
                        nc.scalar.activation(
                            lt[:], ps[:], AF.Identity,
                            bias=bcon_sb[:, j:j + 1])
                        l2.append(lt)
                    # q main
                    for h in range(HC):
                        ps = pspool.tile([128, TS], F32, tag="ps512", bufs=5,
                                         name=f"psbq{s}{h}")
                        for j in range(LJ):
                            nc.tensor.matmul(
                                ps[:],
                                wup_sb[:, JW * j + WQ + HD * h:
                                       JW * j + WQ + HD * (h + 1)],
                                l2[j][:], start=(j == 0),
                                stop=(j == LJ - 1))
                        nc.scalar.activation(
                            qT[h][:, cols], ps[:], AF.Identity,
                            bias=bcon_sb[:, BQ0 + h:BQ0 + h + 1])
                    # q pos: raw up-proj only; the rotate-half operand is
                    # read straight out of PSUM with permuted partition
                    # ranges (sign lives in the sin table)
                    for p in range(2):
                        psr = pspool.tile([128, TS], F32, tag="ps512", bufs=5,
                                          name=f"pspr{s}{p}")
                        for j in range(LJ):
                            nc.tensor.matmul(
                                psr[:],
                                wup_sb[:, JW * j + WP + 128 * p:
                                       JW * j + WP + 128 * (p + 1)],
                                l2[j][:], start=(j == 0),
                                stop=(j == LJ - 1))
                        t3 = bwork.tile([128, TS], F32, tag="qpt", bufs=2,
                                        name=f"qp3{s}{p}")
                        t4 = bwork.tile([128, TS], F32, tag="qpt", bufs=2,
                                        name=f"qp4{s}{p}")
                        nc.vector.scalar_tensor_tensor(
                            t3[:], psr[:], bcon_sb[:, BP0 + 2 * p:
                                                   BP0 + 2 * p + 1],
                            sc2_sb[:, cols], OP.add, OP.mult)
                        for o, i in ((0, 32), (32, 0), (64, 96), (96, 64)):
                            nc.vector.scalar_tensor_tensor(
                                t4[o:o + 32, :], psr[i:i + 32, :],
                                bcon_sb[o:o + 32, BP0 + 2 * p + 1:
                                        BP0 + 2 * p + 2],
                                sc2_sb[o:o + 32, S + TS * s:S + TS * (s + 1)],
                                OP.add, OP.mult)
                        last_ab_dve = nc.vector.tensor_tensor(
                            qpp[p][:, cols], t3[:], t4[:], OP.add)
                    # k main
                    for h in range(HC):
                        ps = pspool.tile([128, TS], F32, tag="ps512", bufs=5,
                                         name=f"psbk{s}{h}")
                        for j in range(LJ):
                            last_ab_mm = nc.tensor.matmul(
                                ps[:],
                                wup_sb[:, JW * j + WK + HD * h:
                                       JW * j + WK + HD * (h + 1)],
                                l2[LJ + j][:], start=(j == 0),
                                stop=(j == LJ - 1))
                        nc.scalar.activation(
                            kT[h][:, cols], ps[:], AF.Identity,
                            bias=bcon_sb[:, BK0 + h:BK0 + h + 1])
                nc.leave_named_scope("AB", _sid, False)

                # o_proj weights take over the 4 wA ring slots for good
                # (span-3's lk tiles are their last other users)
                wol = []
                for oc in range(4):
                    t_ = awork.tile([128, MODEL], BF16, tag="wA", bufs=4,
                                    name=f"wol{oc}")
                    nc.sync.dma_start(
                        out=t_[:],
                        in_=wolp.ap()[:, MODEL * oc:MODEL * (oc + 1)])
                    wol.append(t_)

                _sid = nc.enter_named_scope("Bkv", False)[0]
                # ------- v up-proj + posk unpack (consumes gathered lv) ----
                H = TS // 2
                for r in range(G):
                    latr = bwork.tile([128, 4 * TS], BF16, tag="latB", bufs=2,
                                      name=f"latB{r}")
                    lb_dma = nc.sync.dma_start(
                        out=latr[:],
                        in_=ag_out[128 * r:128 * (r + 1), 0:4 * TS])
                    if r == 0:
                        # scheduler order: AG-gated loads after AB's inputs
                        _dep(lb_dma, last_ab_dma, "latB after AB loads")
                    for tt in range(TS // 128):
                        for h in range(HC):
                            psv = pspool.tile([128, HD], F32, tag="psA",
                                              bufs=3, name=f"psv{r}{tt}{h}")
                            for j in range(LJ):
                                mm = nc.tensor.matmul(
                                    psv[:],
                                    latr[:, TS * j + 128 * tt:
                                         TS * j + 128 * (tt + 1)],
                                    wup_sb[:, JW * j + WV + HD * h:
                                           JW * j + WV + HD * (h + 1)],
                                    start=(j == 0), stop=(j == LJ - 1))
                                if r == 0 and tt == 0 and h == 0 and j == 0:
                                    # keep AG-gated work behind AG-overlapped
                                    # work in the static engine orders
                                    _dep(mm, last_ab_mm, "Bkv after AB")
                            ev = nc.vector.tensor_tensor(
                                v_sb[h][4 * r + tt][:], psv[:],
                                bvb_sb[:, HD * h:HD * (h + 1)], OP.add)
                            if r == 0 and tt == 0 and h == 0:
                                _dep(ev, last_ab_dve, "Bkv DVE after AB")
                    # pos_k -> both halves of posk2, straight from ag_out.
                    # AG layout: rows 128r+[0:64] = dims (tokens 0:256),
                    # rows 128r+[64:128] = dims (tokens 256:512), both at
                    # cols 4TS:4TS+256. posk2 duplicates dims on both
                    # partition halves for the 2-head q-pos packs.
                    for half in range(2):
                        src = ag_out[128 * r + PHD * half:
                                     128 * r + PHD * (half + 1),
                                     4 * TS:4 * TS + H]
                        for p0 in range(2):
                            pk_dma = nc.sync.dma_start(
                                out=posk2[PHD * p0:PHD * (p0 + 1),
                                          TS * r + H * half:
                                          TS * r + H * (half + 1)],
                                in_=src)
                            # pin scheduler order: these AG-gated DMAs must
                            # not precede AB's input loads on any queue
                            _dep(pk_dma, last_ab_dma, "posk2 after AB loads")
                nc.leave_named_scope("Bkv", _sid, False)

                _sid = nc.enter_named_scope("CD", False)[0]

                def ship(ci):
                    # rs_out[ci] -> out_sh via SBUF bounce (DRAM->DRAM DMA is
                    # ~8x slower in the DMA model). Emitted one chunk late so
                    # the SEQ wait on RS_ci can never block the next chunk's
                    # st DMAs.
                    q0, W = CHUNKS[ci]
                    r = W // G
                    rsb = awork.tile([128, MODEL], BF16, tag="xb", bufs=1,
                                     name=f"rsb{ci}")
                    nc.sync.dma_start(out=rsb[0:r, :], in_=rs_out[ci][:])
                    nc.sync.dma_start(
                        out=out_sh.ap()[q0 // G:q0 // G + r, :],
                        in_=rsb[0:r, :])

                # ------ phase C+D: attention + o_proj, chunk-outer ------
                for ci, (q0, W) in enumerate(CHUNKS):
                    attnT = [bwork.tile([128, W], BF16, tag=f"at{h}",
                                        bufs=2, name=f"at{h}_{ci}")
                             for h in range(HC)]
                    for h in range(HC):
                        p, idx = h // 2, h % 2
                        lo, hi = PHD * idx, PHD * (idx + 1)
                        tmax = (q0 + W) // 128 - 1
                        ps_at = pspool.tile([128, W], F32, tag="ps512",
                                            bufs=5, name=f"psat{h}{ci}")
                        ps_sum = pspool.tile([1, W], F32, tag="ps512",
                                             bufs=5, name=f"pssum{h}{ci}")
                        for t in range(tmax + 1):
                            off = 128 * t - q0
                            qlo = max(0, off)
                            kc = 128 * t
                            qs = slice(qlo, W)
                            ps_sc = pspool.tile(
                                [128, W], F32, tag="ps512", bufs=5,
                                name=f"pssc{h}{ci}{t}")
                            nc.tensor.matmul(
                                ps_sc[:, qs], kT[h][:, kc:kc + 128],
                                qT[h][:, q0 + qlo:q0 + W],
                                start=True, stop=False)
                            nc.tensor.matmul(
                                ps_sc[:, qs], posk2[lo:hi, kc:kc + 128],
                                qpp[p][lo:hi, q0 + qlo:q0 + W],
                                start=False, stop=True)
                            pt = bwork.tile([128, W], BF16, tag="pt",
                                            bufs=4, name=f"pt{h}{ci}{t}")
                            nc.scalar.activation(pt[:, qs], ps_sc[:, qs],
                                                 AF.Exp, scale=SCALE)
                            if off >= 0:
                                nc.vector.tensor_tensor(
                                    pt[:, qlo:qlo + 128],
                                    pt[:, qlo:qlo + 128], tri_sb[:],
                                    OP.mult)
                            nc.tensor.matmul(
                                ps_at[:, qs], v_sb[h][t][:], pt[:, qs],
                                start=(t == 0), stop=(t == tmax))
                            nc.tensor.matmul(
                                ps_sum[:, qs], ones_col[:], pt[:, qs],
                                start=(t == 0), stop=(t == tmax))
                        recf = bwork.tile([1, W], F32, tag="recf",
                                          bufs=2, name=f"recf{h}{ci}")
                        nc.vector.reciprocal(recf[:], ps_sum[0:1, :])
                        recb = bwork.tile([1, W], BF16, tag="recb",
                                          bufs=2, name=f"recb{h}{ci}")
                        nc.scalar.copy(recb[:], recf[:])
                        ps_rb = pspool.tile([128, W], F32, tag="psA",
                                            bufs=3, name=f"psrb{h}{ci}")
                        nc.tensor.matmul(ps_rb[:], ones_row[:], recb[:],
                                         start=True, stop=True)
                        rb_sb = bwork.tile([128, W], BF16, tag="rbsb",
                                           bufs=2, name=f"rbsb{h}{ci}")
                        nc.scalar.copy(rb_sb[:], ps_rb[:])
                        nc.vector.tensor_tensor(
                            attnT[h][:], ps_at[:], rb_sb[:], OP.mult)

                    # ---- phase D for this chunk: partial o_proj + RS
                    for tt in range(W // 128):
                        st = bwork.tile([128, MODEL], BF16, tag="st",
                                        bufs=2, name=f"st{ci}{tt}")
                        for oc in range(4):
                            ps = pspool.tile([128, TS], F32, tag="psA",
                                             bufs=3, name=f"psd{ci}{tt}{oc}")
                            for h in range(HC):
                                nc.tensor.matmul(
                                    ps[:],
                                    attnT[h][:, 128 * tt:128 * (tt + 1)],
                                    wol[oc][:, TS * h:TS * (h + 1)],
                                    start=(h == 0), stop=(h == HC - 1))
                            nc.vector.tensor_tensor(
                                st[:, TS * oc:TS * (oc + 1)], ps[:],
                                bob_sb[:, TS * oc:TS * (oc + 1)], OP.add)
                            if ci == len(CHUNKS) - 1:
                                # tail chunk: ship each column slice as its
                                # bias-add lands so the final RS starts ASAP
                                nc.sync.dma_start(
                                    out=rs_in[ci][128 * tt:128 * (tt + 1),
                                                  TS * oc:TS * (oc + 1)],
                                    in_=st[:, TS * oc:TS * (oc + 1)])
                        if ci != len(CHUNKS) - 1:
                            nc.sync.dma_start(
                                out=rs_in[ci][128 * tt:128 * (tt + 1), :],
                                in_=st[:])
                    nc.gpsimd.collective_compute(
                        "ReduceScatter", OP.add,
                        ins=[rs_in[ci].opt()], outs=[rs_out[ci].opt()],
                        replica_groups=groups)
                    if ci > 0:
                        ship(ci - 1)
                ship(len(CHUNKS) - 1)
    nc.leave_named_scope("CD", _sid, False)
    nc.compile()
    return nc


def _host_prep(inputs):
    x = np.asarray(inputs["x"], np.float32)
    w_qkv, b_qkv = inputs["w_qkv"], inputs["b_qkv"]
    w_qup, b_qup = inputs["w_qup"], inputs["b_qup"]
    w_kup, b_kup = inputs["w_kup"], inputs["b_kup"]
    w_vup, b_vup = inputs["w_vup"], inputs["b_vup"]
    w_qpos, b_qpos = inputs["w_qpos"], inputs["b_qpos"]
    w_kpos, b_kpos = inputs["w_kpos"], inputs["b_kpos"]
    w_o, b_o = inputs["w_o"], inputs["b_o"]

    x_flat = x.reshape(T, MODEL)

    # rope tables (position within sequence; same for both batches)
    inv_freq = 1.0 / (THETA ** (np.arange(0, PHD, 2, dtype=np.float32) / PHD))
    pos = np.arange(S, dtype=np.float32)
    freqs = np.outer(pos, inv_freq)
    emb = np.concatenate([freqs, freqs], -1)            # [S, 64]
    cos = np.cos(emb).astype(np.float32)
    sin = np.sin(emb).astype(np.float32)
    sin_signed = np.concatenate([-sin[:, :32], sin[:, 32:]], -1)
    cosT = np.concatenate([cos, cos], 1).T              # [128, S] (2 stacked)
    sinT = np.concatenate([sin_signed, sin_signed], 1).T
    sc2 = np.concatenate([cosT, sinT], 1).astype(BF)    # [128, 2S]

    w_cat = np.concatenate(
        [w_qkv, w_kpos, w_kpos[:, _ROT]], 1).astype(np.float32)  # [2048,1664]
    w_catp = np.ascontiguousarray(
        w_cat.reshape(16, 128, NLT, 128).transpose(1, 2, 0, 3)
        .reshape(128, NLT * 2048)).astype(BF)

    bcat = np.zeros((128, NLT), np.float32)
    for j in range(12):
        bcat[:, j] = b_qkv[128 * j:128 * (j + 1)]
    bcat[0:PHD, 12] = b_kpos
    bcat[PHD:128, 12] = b_kpos[_ROT]

    tri_m = np.triu(np.ones((128, 128), np.float32)).astype(BF)

    bob = np.tile(np.asarray(b_o, np.float32).reshape(1, MODEL) / G,
                  (128, 1)).astype(BF)

    # per-batch xTb: span-major m-major pack of the whole batch
    def pack_xt(x2):                                 # [ntok, MODEL]
        n = x2.shape[0]
        return np.ascontiguousarray(
            x2.reshape(n // TS, TS, 16, 128).transpose(3, 0, 2, 1)
            .reshape(128, (n // TS) * 16 * TS)).astype(BF)

    xTb_g = [pack_xt(x_flat[S * g:S * (g + 1)]) for g in range(B)]

    common = {"w_catp": w_catp, "sc2": sc2, "tri": tri_m, "bob": bob}

    in_maps = []
    for c in range(NC):
        w = c % G
        h0 = HC * w
        cm = slice(HD * h0, HD * (h0 + HC))          # 4-head main cols
        cp = slice(PHD * h0, PHD * (h0 + HC))        # 4-head pos cols
        wq = np.asarray(w_qup[:, cm], np.float32)
        wk = np.asarray(w_kup[:, cm], np.float32)
        wv = np.asarray(w_vup[:, cm], np.float32)
        wp = np.asarray(w_qpos[:, cp], np.float32)   # [512, 256]
        wup_l = np.concatenate([
            np.concatenate([wq[128 * j:128 * (j + 1)],
                            wk[128 * j:128 * (j + 1)],
                            wv[128 * j:128 * (j + 1)],
                            wp[128 * j:128 * (j + 1)]], 1)
            for j in range(LJ)], 1).astype(BF)       # [128, 4*JW]

        # per-core w_o rows (this core's heads), oc-major:
        # col = 2048*oc + 512*h + c'
        wol_l = np.ascontiguousarray(
            np.asarray(w_o[HD * h0:HD * (h0 + HC), :], np.float32)
            .reshape(HC, 128, 4, TS).transpose(1, 2, 0, 3)
            .reshape(128, HC * MODEL)).astype(BF)

        bc = np.zeros((128, BP0 + 4), np.float32)
        bc[:, 0:NLT] = bcat
        for i in range(HC):
            bc[:, BQ0 + i] = b_qup[HD * (h0 + i):HD * (h0 + i + 1)]
            bc[:, BK0 + i] = b_kup[HD * (h0 + i):HD * (h0 + i + 1)]
        for p in range(2):
            bq2 = np.concatenate(
                [b_qpos[PHD * (h0 + 2 * p + i):PHD * (h0 + 2 * p + i + 1)]
                 for i in range(2)])                 # [128]
            bc[:, BP0 + 2 * p] = bq2
            bc[:, BP0 + 2 * p + 1] = np.concatenate(
                [bq2[0:PHD][_ROT], bq2[PHD:128][_ROT]])

        bvb_l = np.tile(np.asarray(b_vup[cm], np.float32).reshape(1, -1),
                        (128, 1)).astype(BF)

        tok = slice(TS * c, TS * (c + 1))
        xT_l = pack_xt(x_flat[tok])                  # [128, 16*TS]

        spos = slice(TS * w, TS * (w + 1))           # positions within batch
        scsh = np.concatenate(
            [cosT[0:PHD, spos], sinT[0:PHD, spos]], 0).astype(np.float32)

        m = {"xT": xT_l, "xTb": xTb_g[c // G], "wup": wup_l, "wolp": wol_l,
             "bcon": bc, "bvb": bvb_l, "sc_sh": scsh}
        m.update(common)
        in_maps.append(m)
    return in_maps


def kernel(**inputs) -> np.ndarray:
    if "nc" not in _CACHE:
        _CACHE["nc"] = _build()
    nc = _CACHE["nc"]
    in_maps = _host_prep({k: np.asarray(v) for k, v in inputs.items()})
    res = run_bass_kernel_spmd(nc, in_maps, list(range(NC))).results
    # core c = (b*G + w) holds, per chunk (q0, W), batch-b tokens
    # [q0 + (W/4)w, q0 + (W/4)(w+1)) in out_sh rows [q0/4, q0/4 + W/4)
    out = np.empty((B, S, MODEL), np.float32)
    for b in range(B):
        for w in range(G):
            sh = res[b * G + w]["out_sh"].astype(np.float32)
            for q0, W in CHUNKS:
                r = W // G
                lo = q0 + r * w
                out[b, lo:lo + r] = sh[q0 // G:q0 // G + r]
    return out
